# revision 1
# baseline (speedup 1.0000x reference)
"""Chamfer loss kernel for Trainium2, 8 NeuronCores (SPMD data-parallel).

Strategy (matches the sharding hint: data-parallel over the selected pairs):
  - Host: dedupe the (batch, seed) pairs in idx (weights = multiplicities).
    Each pair's 2048 x-points are split into two half-units of 8 M-stripes;
    the 2U half-units are distributed round-robin over the 8 cores
    (S = ceil(2U/8) slots per core; with the standard input U=44 -> S=11,
    zero padding waste).
  - For each pair, build matmul operands so the PE computes the full
    distance matrix directly:  P[i,j] = |x_i|^2 + |y_j|^2 - 2 x_i.y_j
    via a K=16 contraction in bf16 hi/lo split (fp32-quality products):
      k 0..2 : xh_d   *  ah_d        (a = -2y, h/l = bf16 hi/lo split)
      k 3..5 : xh_d   *  al_d
      k 6..8 : xl_d   *  ah_d
      k 9..11: xl_d   *  al_d
      k 12/13: rxh/rxl * 1           (rx = |x|^2)
      k 14/15: 1      *  ryh/ryl     (ry = |y|^2)
  - Device per half-unit: 8 M-stripes x 4 N-blocks of [128,512] fp32 in
    PSUM. ScalarE copies each 4-bank PSUM stripe to SBUF fp16 in one op.
    VectorE: rowmin per stripe via a tensor_tensor(min) fold chain
    (2048->1024->512->256) staged + one 3D tensor_reduce per unit; colmin
    via a running tensor_tensor(min) into a [128,2048] accumulator; final
    partition-axis min via PE transposes + fold + 3D tensor_reduce.
  - Output per half-unit: [128, 24] fp32 (8 stripe rowmin columns + 16
    colmin chunk columns). Host combines the two half-units of each pair
    (sum rowmins; elementwise-min then sum colmins), weights by
    multiplicity, divides by num.
"""

import numpy as np
import ml_dtypes
from contextlib import ExitStack

import concourse.bacc as bacc
import concourse.tile as tile
from concourse import mybir
from concourse.bass_utils import run_bass_kernel_spmd

N_CORES = 8
NPTS = 2048
NHALF = 8      # M-stripes per half-unit
NCHUNK = 16    # 128-wide y-chunks for the transpose tail
BF16 = ml_dtypes.bfloat16
F16 = mybir.dt.float16
F32 = mybir.dt.float32
MIN = mybir.AluOpType.min

_BUILD_CACHE = {}


def build_program(n_slots: int, repeats: int = 1):
    """Build + compile the per-core bass program for n_slots half-units."""
    key = (n_slots, repeats)
    if key in _BUILD_CACHE:
        return _BUILD_CACHE[key]

    nc = bacc.Bacc(
        "TRN2", target_bir_lowering=False, debug=False, num_devices=N_CORES
    )
    w_ap = nc.dram_tensor(
        "w", [n_slots, 16, NHALF * 128], mybir.dt.bfloat16, kind="ExternalInput"
    ).ap()
    r_ap = nc.dram_tensor(
        "r", [n_slots, 16, NPTS], mybir.dt.bfloat16, kind="ExternalInput"
    ).ap()
    id_ap = nc.dram_tensor(
        "ident", [128, 128], F16, kind="ExternalInput"
    ).ap()
    o_ap = nc.dram_tensor(
        "o", [n_slots, 128, NHALF + NCHUNK], F32, kind="ExternalOutput"
    ).ap()

    with tile.TileContext(nc) as tc:
        with ExitStack() as ctx:
            const_pool = ctx.enter_context(tc.tile_pool(name="const", bufs=1))
            in_pool = ctx.enter_context(tc.tile_pool(name="inp", bufs=2))
            stripe_pool = ctx.enter_context(tc.tile_pool(name="stripe", bufs=4))
            fold_pool = ctx.enter_context(tc.tile_pool(name="fold", bufs=3))
            acc_pool = ctx.enter_context(tc.tile_pool(name="acc", bufs=2))
            stage_pool = ctx.enter_context(tc.tile_pool(name="stage", bufs=2))
            out_pool = ctx.enter_context(tc.tile_pool(name="outp", bufs=3))
            mm_psum = ctx.enter_context(
                tc.tile_pool(name="mmps", bufs=2, space="PSUM")
            )

            ident = const_pool.tile([128, 128], F16)
            nc.sync.dma_start(ident[:], id_ap[:])

            def body():
                for s in range(n_slots):
                    wt = in_pool.tile([16, NHALF * 128], mybir.dt.bfloat16, tag="wt")
                    rt = in_pool.tile([16, NPTS], mybir.dt.bfloat16, tag="rt")
                    nc.sync.dma_start(wt[:], w_ap[s])
                    nc.sync.dma_start(rt[:], r_ap[s])

                    acc = acc_pool.tile([128, NPTS], F16)
                    outt = out_pool.tile([128, NHALF + NCHUNK], F32)
                    rowstage = stage_pool.tile([128, NHALF, 1024], F16)

                    stripe0 = None
                    for m in range(NHALF):
                        stripe = stripe_pool.tile([128, NPTS], F16)
                        ps = mm_psum.tile([128, 4, 512], F32, tag="ps")
                        for n in range(4):
                            nc.tensor.matmul(
                                ps[:, n, :],
                                lhsT=wt[:, 128 * m : 128 * (m + 1)],
                                rhs=rt[:, 512 * n : 512 * (n + 1)],
                                start=True,
                                stop=True,
                            )
                        nc.scalar.activation(
                            out=stripe[:],
                            in_=ps[:],
                            func=mybir.ActivationFunctionType.Copy,
                        )
                        # rowmin fold level 1 straight into the stage
                        nc.vector.tensor_tensor(
                            rowstage[:, m, :],
                            stripe[:, 0:1024],
                            stripe[:, 1024:2048],
                            MIN,
                        )
                        # colmin accumulate (acc = min of stripes seen)
                        if m == 0:
                            stripe0 = stripe
                        elif m == 1:
                            nc.vector.tensor_tensor(
                                acc[:], stripe0[:], stripe[:], MIN
                            )
                        else:
                            nc.vector.tensor_tensor(
                                acc[:], acc[:], stripe[:], MIN
                            )

                    # combined tail stage: [:,0:8,:] row mins, [:,8:24,:] col
                    cmb = fold_pool.tile([128, NHALF + NCHUNK, 64], F16, tag="cmb")

                    # rowmin tail: fold [128,8,1024] down into cmb
                    rs1 = fold_pool.tile([128, NHALF, 512], F16, tag="rs1")
                    nc.vector.tensor_tensor(
                        rs1[:], rowstage[:, :, 0:512], rowstage[:, :, 512:1024], MIN
                    )
                    rs2 = fold_pool.tile([128, NHALF, 256], F16, tag="rs2")
                    nc.vector.tensor_tensor(
                        rs2[:], rs1[:, :, 0:256], rs1[:, :, 256:512], MIN
                    )
                    rs3 = fold_pool.tile([128, NHALF, 128], F16, tag="rs3")
                    nc.vector.tensor_tensor(
                        rs3[:], rs2[:, :, 0:128], rs2[:, :, 128:256], MIN
                    )
                    nc.vector.tensor_tensor(
                        cmb[:, 0:NHALF, :], rs3[:, :, 0:64], rs3[:, :, 64:128], MIN
                    )

                    # colmin tail: PE transpose acc -> PSUM, ACT copy to
                    # SBUF (frees PSUM + cheaper than a 1x PSUM reduce),
                    # fold into cmb
                    tp = mm_psum.tile([128, NCHUNK, 128], F16, tag="ps")
                    for t in range(NCHUNK):
                        nc.tensor.transpose(
                            tp[:, t, :],
                            acc[:, 128 * t : 128 * (t + 1)],
                            ident[:],
                        )
                    tpc = fold_pool.tile([128, NCHUNK, 128], F16, tag="tpc")
                    nc.scalar.activation(
                        out=tpc[:],
                        in_=tp[:],
                        func=mybir.ActivationFunctionType.Copy,
                    )
                    nc.vector.tensor_tensor(
                        cmb[:, NHALF:, :], tpc[:, :, 0:64], tpc[:, :, 64:128], MIN
                    )

                    # one reduce for both tails -> [128, 24]
                    nc.vector.tensor_reduce(
                        out=outt[:],
                        in_=cmb[:],
                        axis=mybir.AxisListType.X,
                        op=MIN,
                    )
                    nc.sync.dma_start(o_ap[s], outt[:])

            if repeats == 1:
                body()
            else:
                with tc.For_i(0, repeats, 1):
                    body()

    nc.compile()
    _BUILD_CACHE[key] = nc
    return nc


def _split_bf16(x: np.ndarray):
    hi = x.astype(BF16)
    lo = (x - hi.astype(np.float32)).astype(BF16)
    return hi, lo


def make_pair_operands(x: np.ndarray, y: np.ndarray):
    """x: [3, M] fp32 (x-points), y: [3, NPTS] fp32 -> (W [16, M], R [16, NPTS])."""
    m = x.shape[1]
    a = -2.0 * y
    rx = (x * x).sum(axis=0)
    ry = (y * y).sum(axis=0)
    xh, xl = _split_bf16(x)
    ah, al = _split_bf16(a)
    rxh, rxl = _split_bf16(rx)
    ryh, ryl = _split_bf16(ry)

    W = np.empty((16, m), dtype=BF16)
    R = np.empty((16, NPTS), dtype=BF16)
    W[0:3] = xh
    R[0:3] = ah
    W[3:6] = xh
    R[3:6] = al
    W[6:9] = xl
    R[6:9] = ah
    W[9:12] = xl
    R[9:12] = al
    W[12] = rxh
    R[12] = np.ones(NPTS, dtype=BF16)
    W[13] = rxl
    R[13] = np.ones(NPTS, dtype=BF16)
    W[14] = np.ones(m, dtype=BF16)
    R[14] = ryh
    W[15] = np.ones(m, dtype=BF16)
    R[15] = ryl
    return W, R


def prepare_inputs(preds: np.ndarray, gts: np.ndarray, idx: np.ndarray):
    """Dedupe pairs, build per-core input maps for half-unit slots.

    Returns (in_maps, plan, S, num); plan entries are
    (pair_index, count, [(core, slot) half 0, (core, slot) half 1]).
    """
    preds = np.asarray(preds, dtype=np.float32)
    gts = np.asarray(gts, dtype=np.float32)
    idx = np.asarray(idx)
    num = idx.shape[0]

    uniq = {}
    for row in idx:
        key = (int(row[0]), int(row[1]))
        uniq[key] = uniq.get(key, 0) + 1
    pairs = list(uniq.items())  # [((b, s), count)]
    U = len(pairs)
    n_units = 2 * U
    S = (n_units + N_CORES - 1) // N_CORES

    W_all = np.zeros((N_CORES, S, 16, NHALF * 128), dtype=BF16)
    R_all = np.zeros((N_CORES, S, 16, NPTS), dtype=BF16)
    plan = []
    for i, ((b, sd), cnt) in enumerate(pairs):
        x = preds[b, :, :, sd]  # [3, NPTS]
        y = gts[b]              # [3, NPTS]
        W, R = make_pair_operands(x, y)
        locs = []
        for half in range(2):
            u = 2 * i + half
            core, slot = u % N_CORES, u // N_CORES
            W_all[core, slot] = W[:, half * NHALF * 128 : (half + 1) * NHALF * 128]
            R_all[core, slot] = R
            locs.append((core, slot))
        plan.append((i, cnt, locs))
    # padded slots stay zero; their outputs are never read.

    ident = np.eye(128, dtype=np.float16)
    in_maps = [
        {"w": W_all[c], "r": R_all[c], "ident": ident} for c in range(N_CORES)
    ]
    return in_maps, plan, S, num


def finish(results, plan, num):
    total = 0.0
    for _, cnt, locs in plan:
        (c0, s0), (c1, s1) = locs
        o0 = results[c0]["o"][s0]  # [128, 8+16] f32
        o1 = results[c1]["o"][s1]
        rowsum = float(o0[:, 0:NHALF].sum(dtype=np.float64)) + float(
            o1[:, 0:NHALF].sum(dtype=np.float64)
        )
        col = np.minimum(o0[:, NHALF:], o1[:, NHALF:])
        colsum = float(col.sum(dtype=np.float64))
        total += cnt * (rowsum + colsum)
    return np.float32(total / num)


def kernel(preds, gts, idx):
    in_maps, plan, S, num = prepare_inputs(preds, gts, idx)
    nc = build_program(S)
    res = run_bass_kernel_spmd(nc, in_maps, list(range(N_CORES)))
    return finish(res.results, plan, num)



# revision 4
# speedup vs baseline: 4.8763x; 4.8763x over previous
"""Chamfer loss kernel for Trainium2, 8 NeuronCores (SPMD data-parallel).

Strategy (data-parallel over selected pairs, per the sharding hint):
  - Host: dedupe the (batch, seed) pairs in idx (weights = multiplicities).
    Each pair contributes two direction-units (x->nearest-y and
    y->nearest-x); the 2U units are distributed round-robin over 8 cores.
  - Per unit the 2048 query points are sorted into 16 spatial blocks of
    128 (median-cut k-d splits on the widest axis). For each block the
    host selects the C database points nearest to the block's bounding
    box (rank by squared clamp-distance). The device computes the dense
    [128 queries x C candidates] squared-distance tile per block with a
    K=16 matmul (bf16 hi/lo split products, fp32-quality):
      k 0..2 : xh_d * ah_d   (a = -2y)     k 9..11: xl_d * al_d
      k 3..5 : xh_d * al_d                 k 12/13: rxh/rxl * 1
      k 6..8 : xl_d * ah_d                 k 14/15: 1 * ryh/ryl
    then min-reduces over candidates: ACT+DVE evacuate PSUM fp32 -> SBUF
    f16 (split across both engines), a DVE fold + GPSIMD folds halve the
    candidate axis, and one DVE tensor_reduce yields [128, 16] per-query
    mins, DMA'd out.
  - Host: exactness certificate per query: every non-candidate point d
    satisfies dist(q, d) >= depth(q) + rho, where rho is the smallest
    excluded clamp-distance and depth is q's distance to its block's
    bbox boundary (valid when all bbox-interior points are candidates).
    Queries whose device min exceeds the certificate are recomputed
    exactly on host (cKDTree when available). Weighted sum / num.
"""

import numpy as np
import ml_dtypes
from contextlib import ExitStack

import concourse.bacc as bacc
import concourse.tile as tile
from concourse import mybir
from concourse.bass_utils import run_bass_kernel_spmd

N_CORES = 8
NPTS = 2048
NBLK = 16          # query blocks per unit (128 queries each)
CAND = 192         # candidates per block
CPAD = 256         # psum column stride per block (bank-friendly)
NACT = 7           # blocks per half-unit evacuated by ACT (rest: DVE copy)
BF16 = ml_dtypes.bfloat16
F16 = mybir.dt.float16
F32 = mybir.dt.float32
MIN = mybir.AluOpType.min

_BUILD_CACHE = {}


def build_program(n_slots: int, repeats: int = 1):
    """Build + compile the per-core bass program for n_slots units."""
    key = (n_slots, repeats)
    if key in _BUILD_CACHE:
        return _BUILD_CACHE[key]

    c = CAND
    nc = bacc.Bacc(
        "TRN2", target_bir_lowering=False, debug=False, num_devices=N_CORES
    )
    w_ap = nc.dram_tensor(
        "w", [n_slots, 16, NPTS], mybir.dt.bfloat16, kind="ExternalInput"
    ).ap()
    r_ap = nc.dram_tensor(
        "r", [n_slots, 16, NBLK * c], mybir.dt.bfloat16, kind="ExternalInput"
    ).ap()
    o_ap = nc.dram_tensor(
        "o", [n_slots, 128, NBLK], F32, kind="ExternalOutput"
    ).ap()

    with tile.TileContext(nc) as tc:
        with ExitStack() as ctx:
            in_pool = ctx.enter_context(tc.tile_pool(name="inp", bufs=2))
            conv_pool = ctx.enter_context(tc.tile_pool(name="conv", bufs=2))
            fold_pool = ctx.enter_context(tc.tile_pool(name="fold", bufs=2))
            f3_pool = ctx.enter_context(tc.tile_pool(name="f3", bufs=2))
            out_pool = ctx.enter_context(tc.tile_pool(name="outp", bufs=3))
            mm_psum = ctx.enter_context(
                tc.tile_pool(name="mmps", bufs=2, space="PSUM")
            )

            def body():
                for s in range(n_slots):
                    wt = in_pool.tile([16, NPTS], mybir.dt.bfloat16, tag="wt")
                    rt = in_pool.tile([16, NBLK * c], mybir.dt.bfloat16, tag="rt")
                    nc.gpsimd.dma_start(wt[:], w_ap[s])
                    nc.gpsimd.dma_start(rt[:], r_ap[s])

                    outt = out_pool.tile([128, NBLK], F32)
                    f3u = f3_pool.tile([128, NBLK, c // 8], F16, tag="f3")

                    for h in range(2):
                        ps = mm_psum.tile([128, 8, CPAD], F32, tag="ps")
                        for b in range(8):
                            blk = 8 * h + b
                            nc.tensor.matmul(
                                ps[:, b, 0:c],
                                lhsT=wt[:, 128 * blk : 128 * (blk + 1)],
                                rhs=rt[:, c * blk : c * (blk + 1)],
                                start=True,
                                stop=True,
                            )
                        cp = conv_pool.tile([128, 8, c], F16, tag="cp")
                        nc.scalar.activation(
                            out=cp[:, 0:NACT, :],
                            in_=ps[:, 0:NACT, 0:c],
                            func=mybir.ActivationFunctionType.Copy,
                        )
                        nc.vector.tensor_copy(cp[:, NACT:8, :], ps[:, NACT:8, 0:c])
                        f1 = fold_pool.tile([128, 8, c // 2], F16, tag="f1")
                        nc.vector.tensor_tensor(
                            f1[:], cp[:, :, 0 : c // 2], cp[:, :, c // 2 : c], MIN
                        )
                        f2 = fold_pool.tile([128, 8, c // 4], F16, tag="f2")
                        nc.vector.tensor_tensor(
                            f2[:], f1[:, :, 0 : c // 4], f1[:, :, c // 4 : c // 2], MIN
                        )
                        nc.vector.tensor_tensor(
                            f3u[:, 8 * h : 8 * h + 8, :],
                            f2[:, :, 0 : c // 8],
                            f2[:, :, c // 8 : c // 4],
                            MIN,
                        )

                    nc.vector.tensor_reduce(
                        out=outt[:],
                        in_=f3u[:],
                        axis=mybir.AxisListType.X,
                        op=MIN,
                    )
                    nc.gpsimd.dma_start(o_ap[s], outt[:])

            if repeats == 1:
                body()
            else:
                with tc.For_i(0, repeats, 1):
                    body()

    nc.compile()
    _BUILD_CACHE[key] = nc
    return nc


def _split_bf16(x: np.ndarray):
    hi = x.astype(BF16)
    lo = (x - hi.astype(np.float32)).astype(BF16)
    return hi, lo


def _make_w(qs: np.ndarray) -> np.ndarray:
    """qs: [3, 2048] fp32 sorted queries -> W [16, 2048] bf16."""
    n = qs.shape[1]
    rx = (qs * qs).sum(axis=0)
    xh, xl = _split_bf16(qs)
    rxh, rxl = _split_bf16(rx)
    W = np.empty((16, n), dtype=BF16)
    W[0:3] = xh
    W[3:6] = xh
    W[6:9] = xl
    W[9:12] = xl
    W[12] = rxh
    W[13] = rxl
    W[14:16] = np.ones((2, n), dtype=BF16)
    return W


def _make_r(dc: np.ndarray) -> np.ndarray:
    """dc: [NBLK, C, 3] fp32 candidate coords -> R [16, NBLK*C] bf16."""
    y = dc.reshape(-1, 3).T  # [3, NBLK*C]
    a = -2.0 * y
    ry = (y * y).sum(axis=0)
    ah, al = _split_bf16(a)
    ryh, ryl = _split_bf16(ry)
    n = y.shape[1]
    R = np.empty((16, n), dtype=BF16)
    R[0:3] = ah
    R[3:6] = al
    R[6:9] = ah
    R[9:12] = al
    R[12:14] = np.ones((2, n), dtype=BF16)
    R[14] = ryh
    R[15] = ryl
    return R


def _kd_order(Q: np.ndarray) -> np.ndarray:
    """Median-cut widest-axis splits of Q [N,3] into NBLK groups of equal
    size; returns the concatenated index order (block-major)."""
    groups = [np.arange(Q.shape[0])]
    while len(groups) < NBLK:
        new = []
        for g in groups:
            pts = Q[g]
            ax = int(np.argmax(pts.max(0) - pts.min(0)))
            o = g[np.argsort(Q[g, ax], kind="stable")]
            h = len(o) // 2
            new.append(o[:h])
            new.append(o[h:])
        groups = new
    return np.concatenate(groups)


def prepare_inputs(preds: np.ndarray, gts: np.ndarray, idx: np.ndarray):
    """Dedupe pairs, build per-core input maps + certificate metadata.

    Returns (in_maps, plan, S, num). plan entries:
      (cnt, core, slot, Qs [2048,3] f32, D [2048,3] f32,
       rho2 [NBLK] f64, depth [NBLK,128] f64)
    """
    preds = np.asarray(preds, dtype=np.float32)
    gts = np.asarray(gts, dtype=np.float32)
    idx = np.asarray(idx)
    num = idx.shape[0]

    uniq = {}
    for row in idx:
        key = (int(row[0]), int(row[1]))
        uniq[key] = uniq.get(key, 0) + 1
    pairs = list(uniq.items())
    n_units = 2 * len(pairs)
    S = (n_units + N_CORES - 1) // N_CORES

    W_all = np.zeros((N_CORES, S, 16, NPTS), dtype=BF16)
    R_all = np.zeros((N_CORES, S, 16, NBLK * CAND), dtype=BF16)
    plan = []
    u = 0
    for (b, sd), cnt in pairs:
        X = preds[b, :, :, sd].T  # [2048, 3]
        Y = gts[b].T              # [2048, 3]
        for Q, D in ((X, Y), (Y, X)):
            order = _kd_order(Q)
            Qs = Q[order]                          # [2048, 3] block-major
            blocks = Qs.reshape(NBLK, 128, 3)
            lo = blocks.min(axis=1)                # [NBLK, 3]
            hi = blocks.max(axis=1)
            clamped = np.clip(D[None, :, :], lo[:, None, :], hi[:, None, :])
            bbd = ((D[None, :, :] - clamped) ** 2).sum(-1)  # [NBLK, 2048]
            part = np.argpartition(bbd, CAND, axis=1)
            cand = part[:, :CAND]                  # [NBLK, CAND]
            rho2 = np.take_along_axis(bbd, part[:, CAND : CAND + 1], axis=1)[:, 0]
            depth = np.minimum(blocks - lo[:, None, :], hi[:, None, :] - blocks).min(
                axis=2
            )  # [NBLK, 128]

            core, slot = u % N_CORES, u // N_CORES
            W_all[core, slot] = _make_w(Qs.T)
            R_all[core, slot] = _make_r(
                np.take_along_axis(D[None, :, :], cand[:, :, None], axis=1)
            )
            plan.append(
                (cnt, core, slot, Qs, D, rho2.astype(np.float64),
                 depth.astype(np.float64))
            )
            u += 1

    in_maps = [{"w": W_all[c], "r": R_all[c]} for c in range(N_CORES)]
    return in_maps, plan, S, num


def _exact_min_sq(queries: np.ndarray, D: np.ndarray) -> np.ndarray:
    """Exact squared nn distance of each query against D (host fixup)."""
    try:
        from scipy.spatial import cKDTree
    except Exception:
        out = np.empty(queries.shape[0])
        for i in range(0, queries.shape[0], 512):
            q = queries[i : i + 512]
            d2 = ((q[:, None, :] - D[None, :, :]) ** 2).sum(-1)
            out[i : i + 512] = d2.min(axis=1)
        return out
    tree = cKDTree(D)
    dd, _ = tree.query(queries)
    return dd ** 2


def finish(results, plan, num):
    total = 0.0
    for cnt, core, slot, Qs, D, rho2, depth in plan:
        o = results[core]["o"][slot]          # [128, NBLK] f32
        m = o.T.astype(np.float64)            # [NBLK, 128] block-major mins
        cert = (depth + np.sqrt(np.maximum(rho2, 0.0))[:, None]) ** 2
        suspect = (m >= cert * 0.999) | (rho2 <= 0.0)[:, None]
        if suspect.any():
            qs = Qs.reshape(NBLK, 128, 3)[suspect]
            m[suspect] = _exact_min_sq(qs.astype(np.float64), D.astype(np.float64))
        total += cnt * m.sum()
    return np.float32(total / num)


def kernel(preds, gts, idx):
    in_maps, plan, S, num = prepare_inputs(preds, gts, idx)
    nc = build_program(S)
    res = run_bass_kernel_spmd(nc, in_maps, list(range(N_CORES)))
    return finish(res.results, plan, num)
